# revision 5
# baseline (speedup 1.0000x reference)
"""Trainium2 Bass kernel for LocalSparseAttention.

Problem (hardcoded): B=2, S=2048, D=1024, H=16, HD=64, WINDOW=128 (band
|i-j| <= 64), fp32 I/O.

Sharding: 8 cores = 2 batches x 4 head-groups (4 heads each). Each core:
  - qk projection into transposed layout [512, 2048] (head-pair packed)
  - v projection into natural layout, 19 (possibly 64-shifted) seq chunks
  - banded attention at per-pair (256-query) granularity: scores with the
    two heads' K=64 matmuls interleaved (disjoint PE row-groups run
    concurrently), exp on ACT, 0/1 band mask on DVE, AV + softmax
    denominator via ones-augmented v, normalization via PE broadcast
  - output projection -> fp16 partial [2048, 1024]
Host: fp16 casts/transposes in, sum of 4 partials per batch + fused bias
(b_out + b_v @ w_out) out.

Input DMAs are carved by (kt, ns) and ordered useful-first (wqk + the
first 512 xT columns land first) so real qk matmuls start ~8.5us and warm
the HAM clock-gate themselves; small/late tensors (bqk, masks, wout) are
triggered from the Activation engine's HWDGE queue in parallel with the
Sync queue. No dummy warmup.

All matmuls run in fp16 (1 cycle/row on PE, ~3e-4 rel err) with fp32 PSUM
accumulation; softmax exp input stays fp32.
"""
import sys

if "/opt/trn_rl_repo" not in sys.path:
    sys.path.insert(0, "/opt/trn_rl_repo")

import numpy as np

import concourse.bass as bass
import concourse.mybir as mybir
import concourse.tile as tile
from concourse import bacc
from concourse.bass_utils import run_bass_kernel_spmd

B, S, D, H, HD = 2, 2048, 1024, 16, 64
SCALE = HD**-0.5
C_SUB = 4.0  # subtracted from all scores via the mask; cancels in softmax
MASK_NEG = -30000.0

F16 = mybir.dt.float16
F32 = mybir.dt.float32
F32R = mybir.dt.float32r

# 19 key/value chunk offsets: 15 shifted (128c+64) + aligned 0,128,1792,1920
OFFS = [128 * c + 64 for c in range(15)] + [0, 128, 1792, 1920]


def _chunk_pair(i):
    if i == 0:
        return 15, 16
    if i == 15:
        return 17, 18
    return i - 1, i


def _build_pair_masks():
    # variant 0: (first, interior) — c4=0 pair 0
    # variant 1: (interior, interior)
    # variant 2: (interior, last)  — c4=3 pair 1
    m = _build_masks()  # [128, 3(first/int/last), 2(half), 128]
    mp = np.zeros((128, 3, 2, 2, 128), np.float16)
    mp[:, 0, 0] = m[:, 0]
    mp[:, 0, 1] = m[:, 1]
    mp[:, 1, 0] = m[:, 1]
    mp[:, 1, 1] = m[:, 1]
    mp[:, 2, 0] = m[:, 1]
    mp[:, 2, 1] = m[:, 2]
    return mp


def _build_masks():
    kp = np.arange(128)[:, None]
    p = np.arange(128)[None, :]
    masks = np.zeros((128, 3, 2, 128), np.float16)
    for v, shift in enumerate([0, 64, 128]):
        for half in (0, 1):
            w = 128 * half + kp
            valid = np.abs(p + shift - w) <= 64
            masks[:, v, half, :] = valid.astype(np.float16)
    return masks


def _build_program():
    nc = bacc.Bacc("TRN2", debug=False, num_devices=8)

    xT_d = nc.dram_tensor("xT", [D, S], F16, kind="ExternalInput").ap()
    wqk_d = nc.dram_tensor("wqk", [D, 512], F16, kind="ExternalInput").ap()
    wv_d = nc.dram_tensor("wv", [D, 256], F16, kind="ExternalInput").ap()
    wout_d = nc.dram_tensor("wout", [256, D], F16, kind="ExternalInput").ap()
    bqk_d = nc.dram_tensor("bqk", [128, 4], F32, kind="ExternalInput").ap()
    masks_d = nc.dram_tensor("masks", [128, 3, 2, 2, 128], F16,
                             kind="ExternalInput").ap()
    out_d = nc.dram_tensor("out", [S, D], F16, kind="ExternalOutput").ap()

    with tile.TileContext(nc) as tc:
        with (
            tc.tile_pool(name="const", bufs=1) as cpool,
            tc.tile_pool(name="work", bufs=3) as wpool,
            tc.tile_pool(name="expp", bufs=10) as epool,
            tc.tile_pool(name="ysb", bufs=3) as ypool,
            tc.tile_pool(name="ps512", bufs=2, space="PSUM") as ps512,
            tc.tile_pool(name="psv", bufs=2, space="PSUM") as psv,
            tc.tile_pool(name="pssc", bufs=2, space="PSUM") as pssc,
            tc.tile_pool(name="psav", bufs=2, space="PSUM") as psav,
        ):
            # ---- persistent SBUF tensors ----
            xT_sb = cpool.tile([128, 8, S], F16, tag="xT")
            wqk_sb = cpool.tile([128, 8, 512], F16, tag="wqk")
            wv_sb = cpool.tile([128, 8, 256], F16, tag="wv")
            wout_sb = cpool.tile([128, 2, D], F16, tag="wout")
            bqk_sb = cpool.tile([128, 4], F32, tag="bqk")
            masks_sb = cpool.tile([128, 3, 2, 2, 128], F16, tag="masks")
            qk_sb = cpool.tile([128, 4, S], F16, tag="qk")
            v_sb = cpool.tile([128, 19, 4, 65], F16, tag="v")
            aoT_sb = cpool.tile([128, 2, S], F16, tag="aoT")
            ones_sb = cpool.tile([128, 64], F16, tag="ones")
            onescol_sb = cpool.tile([128, 1], F16, tag="onescol")
            negc_sb = cpool.tile([128, 1], F32, tag="negc")

            xT_r = xT_d.rearrange("(ko kp) s -> kp ko s", kp=128)
            wqk_r = wqk_d.rearrange("(ko kp) n -> kp ko n", kp=128)
            wv_r = wv_d.rearrange("(ko kp) n -> kp ko n", kp=128)

            # ---- input DMAs ----
            # ACT HWDGE queue (idle early): small / late-needed tensors.
            nc.scalar.dma_start(out=bqk_sb[:], in_=bqk_d)
            nc.scalar.dma_start(out=masks_sb[:], in_=masks_d)
            nc.scalar.dma_start(
                out=wout_sb[:],
                in_=wout_d.rearrange("(t p) n -> p t n", p=128),
            )
            # Sync queue, priority order: each trigger is ~0.6us of Sync
            # time, so few+large, with the first qk chunk's feed leading.
            nc.sync.dma_start(out=wqk_sb[:, 0:2], in_=wqk_r[:, 0:2])
            nc.sync.dma_start(out=xT_sb[:, 0:2, 0:512],
                              in_=xT_r[:, 0:2, 0:512])
            nc.sync.dma_start(out=wqk_sb[:, 2:5], in_=wqk_r[:, 2:5])
            nc.sync.dma_start(out=xT_sb[:, 2:5, 0:512],
                              in_=xT_r[:, 2:5, 0:512])
            nc.sync.dma_start(out=wqk_sb[:, 5:8], in_=wqk_r[:, 5:8])
            nc.sync.dma_start(out=xT_sb[:, 5:8, 0:512],
                              in_=xT_r[:, 5:8, 0:512])
            nc.sync.dma_start(out=xT_sb[:, :, 512:1024],
                              in_=xT_r[:, :, 512:1024])
            nc.sync.dma_start(out=wv_sb[:], in_=wv_r[:])
            nc.sync.dma_start(out=xT_sb[:, :, 1024:1536],
                              in_=xT_r[:, :, 1024:1536])
            nc.sync.dma_start(out=xT_sb[:, :, 1536:2048],
                              in_=xT_r[:, :, 1536:2048])

            nc.vector.memset(ones_sb[:], 1.0)
            nc.vector.memset(onescol_sb[:], 1.0)
            nc.vector.memset(negc_sb[:], -C_SUB)
            nc.vector.memset(v_sb[:, :, :, 64:65], 1.0)

            # ---- emission helpers ----
            def emit_qk_ns0_ktouter():
                # first qk chunk, kt-outer so matmuls start as soon as the
                # first (wqk, xT) kt-slices land; 4 psum banks open at once
                rrp = [(ps512, "ps512"), (pssc, "pssc"), (psav, "psav"),
                       (ps512, "ps512")]
                tiles = [pool.tile([128, 512], F32, tag=tg, name=f"qk0m{m}")
                         for m, (pool, tg) in enumerate(rrp)]
                for kt in range(8):
                    for m in range(4):
                        nc.tensor.matmul(
                            out=tiles[m][:],
                            lhsT=wqk_sb[:, kt, m * 128:(m + 1) * 128],
                            rhs=xT_sb[:, kt, 0:512],
                            start=(kt == 0),
                            stop=(kt == 7),
                        )
                for m in range(4):
                    scale = SCALE if m < 2 else 1.0
                    nc.scalar.activation(
                        out=qk_sb[:, m, 0:512],
                        in_=tiles[m][:],
                        func=mybir.ActivationFunctionType.Identity,
                        bias=bqk_sb[:, m:m + 1],
                        scale=scale,
                    )

            def emit_qk_ns(ns):
                for m in range(4):
                    scale = SCALE if m < 2 else 1.0
                    ps = ps512.tile([128, 512], F32, tag="ps512")
                    for kt in range(8):
                        nc.tensor.matmul(
                            out=ps[:],
                            lhsT=wqk_sb[:, kt, m * 128:(m + 1) * 128],
                            rhs=xT_sb[:, kt, ns * 512:(ns + 1) * 512],
                            start=(kt == 0),
                            stop=(kt == 7),
                        )
                    nc.scalar.activation(
                        out=qk_sb[:, m, ns * 512:(ns + 1) * 512],
                        in_=ps[:],
                        func=mybir.ActivationFunctionType.Identity,
                        bias=bqk_sb[:, m:m + 1],
                        scale=scale,
                    )

            def emit_v_chunk(c):
                off = OFFS[c]
                ps = psv.tile([128, 256], F32, tag="psv")
                for kt in range(8):
                    nc.tensor.matmul(
                        out=ps[:],
                        lhsT=xT_sb[:, kt, off:off + 128],
                        rhs=wv_sb[:, kt, :],
                        start=(kt == 0),
                        stop=(kt == 7),
                    )
                nc.scalar.copy(
                    out=v_sb[:, c, :, 0:64],
                    in_=ps[:].rearrange("p (h d) -> p h d", h=4),
                )

            def emit_scores_pair(c4, hp, pair):
                # scores + exp for both heads, one ii-pair (256 queries).
                # hh=0 lives in PE rows 0-63, hh=1 in rows 64-127: the
                # half-interleaved order lets consecutive matmuls execute
                # concurrently in disjoint row-groups.
                if c4 == 0 and pair == 0:
                    pv = 0
                elif c4 == 3 and pair == 1:
                    pv = 2
                else:
                    pv = 1
                scs = {
                    0: pssc.tile([128, 2, 2, 128], F32, tag="pssc",
                                 name="sc_h0"),
                    1: pssc.tile([128, 2, 2, 128], F32, tag="pssc",
                                 name="sc_h1"),
                }
                for iw in range(2):
                    i = c4 * 4 + pair * 2 + iw
                    cA, cB = _chunk_pair(i)
                    for half, cc in enumerate((cA, cB)):
                        off = OFFS[cc]
                        for hh in range(2):
                            po = hh * 64
                            nc.tensor.matmul(
                                out=scs[hh][:, iw, half, :],
                                lhsT=qk_sb[po:po + 64, 2 + hp,
                                           off:off + 128],
                                rhs=qk_sb[po:po + 64, hp,
                                          i * 128:(i + 1) * 128],
                                start=(iw == 0 and half == 0),
                                stop=(iw == 1 and half == 1),
                            )
                ex = {}
                for hh in range(2):
                    ex[hh] = epool.tile([128, 2, 2, 128], F16, tag="exp",
                                        name=f"ex{hh}")
                    nc.scalar.activation(
                        out=ex[hh][:],
                        in_=scs[hh][:],
                        func=mybir.ActivationFunctionType.Exp,
                        bias=negc_sb[:],
                    )
                    nc.vector.tensor_mul(
                        out=ex[hh][:],
                        in0=ex[hh][:],
                        in1=masks_sb[:, pv],
                    )
                return ex

            def emit_av_norm_pair(c4, hp, pair, ex):
                av = {}
                for hh in range(2):
                    h = 2 * hp + hh
                    avt = psav.tile([128, 2, 128], F32, tag="psav",
                                    name=f"avt{hh}")
                    av[hh] = avt
                    mwidth = 65 if hh == 0 else 64
                    outsl = slice(0, 65) if hh == 0 else slice(64, 128)
                    for iw in range(2):
                        i = c4 * 4 + pair * 2 + iw
                        cA, cB = _chunk_pair(i)
                        for half, cc in enumerate((cA, cB)):
                            nc.tensor.matmul(
                                out=avt[outsl, iw, :],
                                lhsT=v_sb[:, cc, h, 0:mwidth],
                                rhs=ex[hh][:, iw, half, :],
                                start=(iw == 0 and half == 0),
                                stop=(iw == 1 and half == 1),
                            )
                    if hh == 1:
                        # odd-head denominators: two N=256 matmuls into
                        # partition 0 (disjoint from the data rows)
                        for half in range(2):
                            nc.tensor.matmul(
                                out=avt[0:1, :, :],
                                lhsT=onescol_sb[:],
                                rhs=ex[hh][:, :, half, :],
                                start=(half == 0),
                                stop=(half == 1),
                            )

                # normalization: denoms -> SBUF f16, PE broadcast,
                # approx-reciprocal on the broadcast, multiply
                den = wpool.tile([65, 256], F16, tag="den")
                nc.scalar.copy(
                    out=den[64:65, :],
                    in_=av[0][64:65, :, :].rearrange("p a b -> p (a b)"),
                )
                nc.scalar.copy(
                    out=den[0:1, :],
                    in_=av[1][0:1, :, :].rearrange("p a b -> p (a b)"),
                )
                bc = ps512.tile([128, 512], F32, tag="ps512")
                nc.tensor.matmul(
                    out=bc[0:64, 0:256], lhsT=ones_sb[64:65, :],
                    rhs=den[64:65, :], start=True, stop=True,
                )
                nc.tensor.matmul(
                    out=bc[64:128, 0:256], lhsT=ones_sb[0:1, :],
                    rhs=den[0:1, :], start=True, stop=True,
                )
                bcs = wpool.tile([128, 256], F32, tag="bcs")
                nc.vector.reciprocal_approx_fast(out=bcs[:], in_=bc[:, 0:256])
                sl = slice(c4 * 512 + pair * 256, c4 * 512 + pair * 256 + 256)
                nc.vector.tensor_mul(
                    out=aoT_sb[0:64, hp, sl],
                    in0=av[0][0:64, :, :].rearrange("p a b -> p (a b)"),
                    in1=bcs[0:64, :],
                )
                nc.vector.tensor_mul(
                    out=aoT_sb[64:128, hp, sl],
                    in0=av[1][64:128, :, :].rearrange("p a b -> p (a b)"),
                    in1=bcs[64:128, :],
                )

            def emit_outproj_st(st):
                for nn in range(2):
                    ps = ps512.tile([128, 512], F32, tag="ps512")
                    for hp2 in range(2):
                        nc.tensor.matmul(
                            out=ps[:],
                            lhsT=aoT_sb[:, hp2, st * 128:(st + 1) * 128],
                            rhs=wout_sb[:, hp2, nn * 512:(nn + 1) * 512],
                            start=(hp2 == 0),
                            stop=(hp2 == 1),
                        )
                    ysb = ypool.tile([128, 512], F16, tag="ysb")
                    if (st * 2 + nn) % 2 == 0:
                        nc.scalar.copy(out=ysb[:], in_=ps[:])
                    else:
                        nc.vector.tensor_copy(out=ysb[:], in_=ps[:])
                    nc.sync.dma_start(
                        out=out_d[st * 128:(st + 1) * 128,
                                  nn * 512:(nn + 1) * 512],
                        in_=ysb[:],
                    )

            # ---- emission schedule: per-pair scores -> filler -> AV so
            # the PE has independent work while ACT runs exp; projections
            # and outproj tiles are the fillers ----
            emit_qk_ns0_ktouter()
            emit_qk_ns(1)

            # (c4, hp, pair) -> list of filler ops emitted between the
            # pair's scores and its AV ("v", c) or ("o", st) or ("qk", ns)
            SEQ = [
                ((0, 0, 0), [("v", 15), ("v", 16), ("v", 0), ("v", 1)]),
                ((0, 0, 1), [("v", 2), ("v", 3)]),
                ((0, 1, 0), [("v", 4)]),
                ((0, 1, 1), [("v", 5)]),
                (None, [("qk", 2)]),
                ((1, 0, 0), [("v", 6)]),
                ((1, 0, 1), [("v", 7)]),
                ((1, 1, 0), [("o", 0)]),
                ((1, 1, 1), [("o", 1)]),
                (None, [("qk", 3)]),
                ((2, 0, 0), [("v", 8), ("v", 9)]),
                ((2, 0, 1), [("v", 10), ("v", 11)]),
                ((2, 1, 0), [("o", 2)]),
                ((2, 1, 1), [("o", 3)]),
                (None, [("v", 12), ("v", 13), ("v", 14), ("v", 17),
                        ("v", 18)]),
                ((3, 0, 0), [("o", 4), ("o", 8)]),
                ((3, 0, 1), [("o", 5), ("o", 9)]),
                ((3, 1, 0), [("o", 6), ("o", 10)]),
                ((3, 1, 1), [("o", 7), ("o", 11), ("o", 12)]),
                (None, [("o", 13), ("o", 14), ("o", 15)]),
            ]

            def emit_filler(kind, arg):
                if kind == "v":
                    emit_v_chunk(arg)
                elif kind == "o":
                    emit_outproj_st(arg)
                else:
                    emit_qk_ns(arg)

            for blk, fillers in SEQ:
                if blk is None:
                    for kind, arg in fillers:
                        emit_filler(kind, arg)
                    continue
                c4, hp, pair = blk
                ex = emit_scores_pair(c4, hp, pair)
                for kind, arg in fillers:
                    emit_filler(kind, arg)
                emit_av_norm_pair(c4, hp, pair, ex)

    nc.compile()
    return nc


_NC = None


def _get_program():
    global _NC
    if _NC is None:
        _NC = _build_program()
    return _NC


def _make_in_maps(x, w_qkv, b_qkv, w_out):
    masks = _build_pair_masks()

    in_maps = []
    for c in range(8):
        b, hg = divmod(c, 4)
        cq = 256 * hg
        wqk = np.concatenate(
            [w_qkv[:, cq:cq + 256], w_qkv[:, 1024 + cq:1024 + cq + 256]],
            axis=1,
        ).astype(np.float16)
        bqk = np.empty((128, 4), np.float32)
        bqk[:, 0] = b_qkv[cq:cq + 128] * SCALE
        bqk[:, 1] = b_qkv[cq + 128:cq + 256] * SCALE
        bqk[:, 2] = b_qkv[1024 + cq:1024 + cq + 128]
        bqk[:, 3] = b_qkv[1024 + cq + 128:1024 + cq + 256]
        in_maps.append({
            "xT": np.ascontiguousarray(x[b].T).astype(np.float16),
            "wqk": wqk,
            "wv": w_qkv[:, 2048 + cq:2048 + cq + 256].astype(np.float16),
            "wout": w_out[cq:cq + 256, :].astype(np.float16),
            "bqk": bqk,
            "masks": masks,
        })
    return in_maps


def kernel(x, w_qkv, b_qkv, w_out, b_out):
    x = np.asarray(x, np.float32)
    w_qkv = np.asarray(w_qkv, np.float32)
    b_qkv = np.asarray(b_qkv, np.float32)
    w_out = np.asarray(w_out, np.float32)
    b_out = np.asarray(b_out, np.float32)

    in_maps = _make_in_maps(x, w_qkv, b_qkv, w_out)
    nc = _get_program()
    res = run_bass_kernel_spmd(nc, in_maps, list(range(8)))

    b_v = b_qkv[2048:]
    bias_all = b_out + b_v @ w_out  # folds the (untracked) v-bias
    y = np.empty((B, S, D), np.float32)
    for b in range(B):
        acc = np.zeros((S, D), np.float32)
        for hg in range(4):
            acc += res.results[4 * b + hg]["out"].astype(np.float32)
        y[b] = acc + bias_all
    return y
